# revision 15
# baseline (speedup 1.0000x reference)
"""ChatGLM3 decoder layer on 8 Trainium2 NeuronCores (tensor-parallel).

Sharding (TP-8, per hint):
  - attention: 4 query heads per core; KV head g = core//4 replicated in groups of 4
  - wqkv rows / wo columns sharded accordingly; AllReduce after wo (on device,
    chunked over 4x512-token blocks to overlap with MLP compute)
  - MLP: ffn dim sharded 1712/core (padded to 1792 for 128-alignment),
    paired a/b halves co-located for SwiGLU; second reduction done on host
    (partial outputs summed during unshard)
  - RMSNorm weights folded into the following matmul weights host-side;
    per-token inv-rms applied on device.

All big matmuls run in float32r (TF32-like: 8-bit exp / 11-bit mantissa,
full fp32 PSUM accumulation) at bf16 speed. Activations are feature-major
(x^T layout) throughout so no on-device transposes are needed except
v (16 small PE transposes) -- scores are computed as scoresT = k^T.T @ q^T
with softmax-sum via ones-matmul over the partition axis and division by
the denominator deferred past the V matmul.
"""

import math
from contextlib import ExitStack

import numpy as np

import concourse.bass as bass
import concourse.bacc as bacc
import concourse.mybir as mybir
import concourse.tile as tile
import concourse.bass_utils as bass_utils
from concourse.masks import make_identity

P = 128
B, S, H = 2, 1024, 4096
T = B * S                    # 2048 tokens
HT = H // P                  # 32 feature tiles
NH, NKV, D = 32, 2, 128
FFN = 13696
F_SH = FFN // 8              # 1712 ffn dims per core
FP_SH = 1792                 # padded to 14*128
FT = FP_SH // P              # 14
QH = NH // 8                 # 4 query heads per core
EPS = 1e-5
ROPE_BASE = 10000.0
N_CORES = 8
NJ = 4                       # 512-token chunks (AllReduce granularity)
CHUNK = T // NJ              # 512
HYPERS = [(0, 2), (2, 4)]    # nj ranges per MLP hyper-chunk (1024 tokens each)

dt = mybir.dt
AF = mybir.ActivationFunctionType
OP = mybir.AluOpType

_CACHE = {}


def _round_tf32(x):
    """Round fp32 to float32r (11-bit mantissa, low 12 bits zero), RNE."""
    u = np.ascontiguousarray(x, dtype=np.float32).view(np.uint32)
    low = u & 0xFFF
    half = np.uint32(0x800)
    r = (u >> 12) + ((low > half) | ((low == half) & ((u >> 12) & 1))).astype(np.uint32)
    return (r << 12).view(np.float32)


def _build_program():
    nc = bacc.Bacc("TRN2", target_bir_lowering=False, debug=False,
                   num_devices=N_CORES)

    io = {}

    def inp(name, shape, dtype):
        io[name] = nc.dram_tensor(name, shape, dtype, kind="ExternalInput").ap()
        return io[name]

    inp("hidT", [H, T], dt.float32r)         # hidden_states^T (tf32-rounded)
    inp("cosT", [P, T], dt.float32)          # rope cos, rows duplicated
    inp("sinT", [P, T], dt.float32)
    inp("maskT", [P, 4 * CHUNK], dt.float32)  # 4 shifted causal masks
    inp("wqkvT", [H, 768], dt.float32r)      # (q4 + k + v) rows, pre-transposed
    inp("bqkvT", [P, 6], dt.float32)         # bias, per row-tile columns
    inp("woT", [512, H], dt.float32r)        # wo[:, shard]^T
    inp("w1T", [H, 2 * FP_SH], dt.float32r)  # [a(1792) b(1792)] columns
    inp("w2T", [FP_SH, H], dt.float32r)
    outT = nc.dram_tensor("outT", [H, T], dt.float32, kind="ExternalOutput").ap()

    with tile.TileContext(nc) as tc:
        _emit(nc, tc, io, outT)
    nc.compile()
    return nc


def _emit(nc, tc, io, outT):
    hidT, cosT, sinT, maskT = io["hidT"], io["cosT"], io["sinT"], io["maskT"]
    wqkvT, bqkvT, woT, w1T, w2T = (io["wqkvT"], io["bqkvT"], io["woT"],
                                   io["w1T"], io["w2T"])
    f32, f32r = dt.float32, dt.float32r

    with ExitStack() as ctx:
        const = ctx.enter_context(tc.tile_pool(name="const", bufs=1))
        ident_f = const.tile([P, P], f32)
        make_identity(nc, ident_f)
        ident = const.tile([P, P], f32r)
        nc.vector.tensor_copy(ident[:], ident_f[:])
        ones_f = const.tile([P, 1], f32)
        nc.any.memset(ones_f[:], 1.0)
        ones_col = const.tile([P, 1], f32r)
        nc.vector.tensor_copy(ones_col[:], ones_f[:])
        ones_rf = const.tile([1, P], f32)
        nc.any.memset(ones_rf[:], 1.0)
        ones_row = const.tile([1, P], f32r)
        nc.vector.tensor_copy(ones_row[:], ones_rf[:])
        bq_sb = const.tile([P, 6], f32)
        nc.sync.dma_start(bq_sb[:], bqkvT[:])
        eps1 = const.tile([1, 1], f32)
        nc.any.memset(eps1[:], EPS)

        dram = ctx.enter_context(tc.tile_pool(name="dram", bufs=1, space="DRAM"))
        arin = [dram.tile([H, CHUNK], f32, name=f"arin{j}") for j in range(NJ)]
        arout = [dram.tile([H, CHUNK], f32, name=f"arout{j}",
                           addr_space="Shared") for j in range(NJ)]
        hm_dram = dram.tile([H, T], f32)
        h_dram = dram.tile([FP_SH, T], f32r)

        with ExitStack() as s1:
            # alive phases 1-4: post-rope q/k (fp32r feature-major) + v tokens
            qkp = s1.enter_context(tc.tile_pool(name="qkp", bufs=1))
            qk_r = [qkp.tile([P, T], f32r, tag=f"qk{i}", name=f"qk{i}")
                    for i in range(5)]
            vtok = qkp.tile([P, 16, P], f32r, tag="vtok")

            # ---------- phase 1+2: qkv matmul, rmsnorm1, rope (per chunk) ----
            with ExitStack() as s1a:
                wq_pool = s1a.enter_context(tc.tile_pool(name="wqkv", bufs=4))
                work = s1a.enter_context(tc.tile_pool(name="p1work", bufs=3))
                rp = s1a.enter_context(tc.tile_pool(name="p1rope", bufs=2))
                qf_pool = s1a.enter_context(tc.tile_pool(name="p1qf", bufs=2))
                ps1 = s1a.enter_context(
                    tc.tile_pool(name="p1ps", bufs=1, space="PSUM"))
                psq = s1a.enter_context(
                    tc.tile_pool(name="p1psq", bufs=1, space="PSUM"))

                for nj in range(NJ):
                    c0 = CHUNK * nj
                    ss = ps1.tile([1, CHUNK], f32, tag="ssbc")
                    qps = [psq.tile([P, CHUNK], f32, tag=f"qp{m}",
                                    name=f"qp{m}") for m in range(6)]
                    for kt in range(HT):
                        hr = work.tile([P, CHUNK], f32r, tag="hr")
                        nc.sync.dma_start(
                            hr[:],
                            hidT[P * kt:P * (kt + 1), c0:c0 + CHUNK])
                        wq = wq_pool.tile([P, 768], f32r, tag="wq")
                        nc.sync.dma_start(
                            wq[:], wqkvT[P * kt:P * (kt + 1), :])
                        sq = work.tile([P, CHUNK], f32r, tag="sq")
                        nc.vector.tensor_mul(sq[:], hr.bitcast(f32)[:],
                                             hr.bitcast(f32)[:])
                        nc.tensor.matmul(ss[:], ones_col[:], sq[:],
                                         start=(kt == 0), stop=(kt == HT - 1))
                        for m in range(6):
                            nc.tensor.matmul(
                                qps[m][:], wq[:, P * m:P * (m + 1)],
                                hr[:], start=(kt == 0), stop=(kt == HT - 1))
                    rms1 = work.tile([1, CHUNK], f32, tag="rms1")
                    nc.scalar.activation(rms1[:], ss[:], AF.Sqrt,
                                         bias=eps1[:], scale=1.0 / H)
                    inv1 = work.tile([1, CHUNK], f32r, tag="inv1")
                    with nc.allow_low_precision(reason="feeds tf32 matmul"):
                        nc.vector.reciprocal(inv1[:], rms1[:])
                    bc = ps1.tile([P, CHUNK], f32, tag="ssbc", name="bc")
                    nc.tensor.matmul(bc[:], ones_row[:], inv1[:],
                                     start=True, stop=True)
                    bc_sb = work.tile([P, CHUNK], f32, tag="bc_sb")
                    nc.vector.tensor_copy(bc_sb[:], bc[:])
                    qf = [qf_pool.tile([P, CHUNK], f32, tag=f"qf{m}",
                                       name=f"qf{m}") for m in range(6)]
                    for m in range(6):
                        nc.vector.tensor_mul(qf[m][:], qps[m][:], bc_sb[:])
                        nc.vector.tensor_scalar_add(qf[m][:], qf[m][:],
                                                    bq_sb[:, m:m + 1])
                    # rope on this chunk for q0..q3, k
                    cos_c = rp.tile([P, CHUNK], f32, tag="cos")
                    sin_c = rp.tile([P, CHUNK], f32, tag="sin")
                    nc.sync.dma_start(cos_c[:], cosT[:, c0:c0 + CHUNK])
                    nc.sync.dma_start(sin_c[:], sinT[:, c0:c0 + CHUNK])
                    for i in range(5):
                        src = qf[i]
                        dstt = qk_r[i]
                        ta = rp.tile([64, CHUNK], f32, tag="ropeA")
                        tb = rp.tile([64, CHUNK], f32, tag="ropeB")
                        nc.vector.tensor_mul(ta[:], src[:64, :], cos_c[:64, :])
                        nc.vector.tensor_mul(tb[:], src[64:, :], sin_c[64:, :])
                        nc.vector.tensor_sub(dstt[:64, c0:c0 + CHUNK],
                                             ta[:], tb[:])
                        nc.vector.tensor_mul(ta[:], src[64:, :], cos_c[64:, :])
                        nc.vector.tensor_mul(tb[:], src[:64, :], sin_c[:64, :])
                        nc.vector.tensor_add(dstt[64:, c0:c0 + CHUNK],
                                             ta[:], tb[:])
                    # v: cast + transpose to token-major (4 token tiles/chunk)
                    v_c = work.tile([P, CHUNK], f32r, tag="v_c")
                    nc.vector.tensor_copy(v_c[:], qf[5][:])
                    for loc in range(4):
                        pt = ps1.tile([P, P], f32r, tag="vt")
                        nc.tensor.transpose(pt[:],
                                            v_c[:, P * loc:P * (loc + 1)],
                                            ident[:])
                        nc.vector.tensor_copy(
                            vtok[:, 4 * nj + loc, :],
                            pt.bitcast(f32)[:])

            # ---------------- phase 3: attention ----------------
            with ExitStack() as s3:
                att_pool = s3.enter_context(tc.tile_pool(name="attp", bufs=1))
                attn_s = [att_pool.tile([P, T], f32r, tag=f"attn{h}",
                                        name=f"attn{h}") for h in range(QH)]
                m3 = s3.enter_context(tc.tile_pool(name="p3m", bufs=1))
                mask_sb = m3.tile([P, 4 * CHUNK], f32, tag="mask")
                nc.sync.dma_start(mask_sb[:], maskT[:])
                s3w_stack = ExitStack()
                w3 = s3w_stack.enter_context(tc.tile_pool(name="p3w", bufs=3))
                expp = s3w_stack.enter_context(
                    tc.tile_pool(name="p3exp", bufs=10))
                psA = s3w_stack.enter_context(
                    tc.tile_pool(name="p3ps", bufs=2, space="PSUM"))
                TQJ = S // CHUNK  # 2 tq chunks per batch
                for b in range(B):
                    for h in range(QH):
                        q_t = qk_r[h]
                        for j in range(TQJ):
                            tq0 = b * S + j * CHUNK
                            n_tk = 4 * (j + 1)
                            ps_den = psA.tile([1, CHUNK], f32, tag="den")
                            ps_att = psA.tile([P, CHUNK], f32, tag="att")
                            for i in range(n_tk):
                                ps_s = psA.tile([P, CHUNK], f32, tag="sc")
                                nc.tensor.matmul(
                                    ps_s[:],
                                    qk_r[4][:, b * S + P * i:
                                            b * S + P * (i + 1)],
                                    q_t[:, tq0:tq0 + CHUNK],
                                    start=True, stop=True)
                                ex = expp.tile([P, CHUNK], f32r, tag="exp")
                                nc.scalar.activation(ex[:], ps_s[:], AF.Exp)
                                if i >= 4 * j:  # diagonal block: mask
                                    o = i - 4 * j
                                    nc.vector.tensor_mul(
                                        ex[:], ex.bitcast(f32)[:],
                                        mask_sb[:, o * CHUNK:(o + 1) * CHUNK])
                                nc.tensor.matmul(ps_den[:], ones_col[:], ex[:],
                                                 start=(i == 0),
                                                 stop=(i == n_tk - 1))
                                nc.tensor.matmul(ps_att[:],
                                                 vtok[:, 8 * b + i, :], ex[:],
                                                 start=(i == 0),
                                                 stop=(i == n_tk - 1))
                            rec = w3.tile([1, CHUNK], f32r, tag="rec")
                            with nc.allow_low_precision(reason="tf32 bcast"):
                                nc.vector.reciprocal(rec[:], ps_den[:])
                            ps_bc = psA.tile([P, CHUNK], f32, tag="attbc")
                            nc.tensor.matmul(ps_bc[:], ones_row[:], rec[:],
                                             start=True, stop=True)
                            rb_sb = w3.tile([P, CHUNK], f32, tag="rb_sb")
                            nc.vector.tensor_copy(rb_sb[:], ps_bc[:])
                            nc.vector.tensor_mul(
                                attn_s[h][:, tq0:tq0 + CHUNK],
                                ps_att[:], rb_sb[:])

                s3w_stack.close()
                # ---------- phase 4: wo partial + chunked AllReduce ----------
                with ExitStack() as s4:
                    wo_pool = s4.enter_context(tc.tile_pool(name="wo", bufs=1))
                    wo_sb = wo_pool.tile([P, 4, H], f32r)
                    nc.sync.dma_start(
                        wo_sb[:], woT.rearrange("(kf p) m -> p kf m", p=P))
                    ps4 = s4.enter_context(
                        tc.tile_pool(name="p4ps", bufs=4, space="PSUM"))
                    ev4 = s4.enter_context(tc.tile_pool(name="p4ev", bufs=4))
                    for nj in range(NJ):
                        for m in range(HT):
                            pp = ps4.tile([P, CHUNK], f32, tag="pp")
                            for kf in range(4):
                                nc.tensor.matmul(
                                    pp[:], wo_sb[:, kf, P * m:P * (m + 1)],
                                    attn_s[kf][:,
                                               CHUNK * nj:CHUNK * (nj + 1)],
                                    start=(kf == 0), stop=(kf == 3))
                            ev = ev4.tile([P, CHUNK], f32, tag="ev")
                            nc.vector.tensor_copy(ev[:], pp[:])
                            nc.sync.dma_start(arin[nj][P * m:P * (m + 1), :],
                                              ev[:])
                        nc.gpsimd.collective_compute(
                            "AllReduce", OP.add,
                            replica_groups=[list(range(N_CORES))],
                            ins=[arin[nj].opt()], outs=[arout[nj].opt()])

        # ---------------- phase 6: residual + rmsnorm2 ----------------
        with ExitStack() as s2:
            bc2p = s2.enter_context(tc.tile_pool(name="bc2p", bufs=1))
            bcast2 = bc2p.tile([P, T], f32, tag="bcast2")
            with ExitStack() as s6:
                w6 = s6.enter_context(tc.tile_pool(name="p6work", bufs=3))
                ps6 = s6.enter_context(
                    tc.tile_pool(name="p6ps", bufs=2, space="PSUM"))
                for nj in range(NJ):
                    ss2 = ps6.tile([1, CHUNK], f32, tag="ss2")
                    for kt in range(HT):
                        hl = w6.tile([P, CHUNK], f32r, tag="hl")
                        nc.sync.dma_start(
                            hl[:], hidT[P * kt:P * (kt + 1),
                                        CHUNK * nj:CHUNK * (nj + 1)])
                        al = w6.tile([P, CHUNK], f32, tag="al")
                        nc.sync.dma_start(al[:],
                                          arout[nj][P * kt:P * (kt + 1), :])
                        hm = w6.tile([P, CHUNK], f32, tag="hm")
                        nc.vector.tensor_add(hm[:], hl.bitcast(f32)[:], al[:])
                        nc.sync.dma_start(
                            hm_dram[P * kt:P * (kt + 1),
                                    CHUNK * nj:CHUNK * (nj + 1)], hm[:])
                        sq2 = w6.tile([P, CHUNK], f32r, tag="sq2")
                        nc.vector.tensor_mul(sq2[:], hm[:], hm[:])
                        nc.tensor.matmul(ss2[:], ones_col[:], sq2[:],
                                         start=(kt == 0), stop=(kt == HT - 1))
                    rms2 = w6.tile([1, CHUNK], f32, tag="rms2")
                    nc.scalar.activation(rms2[:], ss2[:], AF.Sqrt,
                                         bias=eps1[:], scale=1.0 / H)
                    inv2 = w6.tile([1, CHUNK], f32r, tag="inv2")
                    with nc.allow_low_precision(reason="feeds tf32 matmul"):
                        nc.vector.reciprocal(inv2[:], rms2[:])
                    bc2 = ps6.tile([P, CHUNK], f32, tag="bc2")
                    nc.tensor.matmul(bc2[:], ones_row[:], inv2[:],
                                     start=True, stop=True)
                    nc.vector.tensor_copy(
                        bcast2[:, CHUNK * nj:CHUNK * (nj + 1)], bc2[:])

            # ---------------- phases 7/8 per hyper: MLP ----------------
            for hyp, (nj_lo, nj_hi) in enumerate(HYPERS):
                HW_ = CHUNK * (nj_hi - nj_lo)   # 1024
                t0 = CHUNK * nj_lo
                NB = HW_ // 512
                with ExitStack() as s7:
                    yp = s7.enter_context(tc.tile_pool(name="yr", bufs=1))
                    y_r = yp.tile([P, HT, HW_], f32r, tag="y_r")
                    w7 = s7.enter_context(tc.tile_pool(name="p7w", bufs=3))
                    for njl in range(nj_lo, nj_hi):
                        for kt in range(HT):
                            hmb = w7.tile([P, CHUNK], f32, tag="hmb")
                            nc.sync.dma_start(
                                hmb[:],
                                hm_dram[P * kt:P * (kt + 1),
                                        CHUNK * njl:CHUNK * (njl + 1)])
                            nc.vector.tensor_mul(
                                y_r[:, kt, CHUNK * (njl - nj_lo):
                                    CHUNK * (njl - nj_lo + 1)],
                                hmb[:],
                                bcast2[:, CHUNK * njl:CHUNK * (njl + 1)])
                    wst = s7.enter_context(tc.tile_pool(name="w1st", bufs=6))
                    ps7 = s7.enter_context(
                        tc.tile_pool(name="p7ps", bufs=2, space="PSUM"))
                    for t in range(FT):
                        ps_a = [ps7.tile([P, 512], f32, tag=f"psa{nb}",
                                         name=f"psa{nb}") for nb in range(NB)]
                        ps_b = [ps7.tile([P, 512], f32, tag=f"psb{nb}",
                                         name=f"psb{nb}") for nb in range(NB)]
                        for kt in range(HT):
                            wa = wst.tile([P, P], f32r, tag="wa")
                            nc.sync.dma_start(
                                wa[:], w1T[P * kt:P * (kt + 1),
                                           P * t:P * (t + 1)])
                            wb = wst.tile([P, P], f32r, tag="wb")
                            nc.sync.dma_start(
                                wb[:], w1T[P * kt:P * (kt + 1),
                                           FP_SH + P * t:FP_SH + P * (t + 1)])
                            for nb in range(NB):
                                rhs = y_r[:, kt, 512 * nb:512 * (nb + 1)]
                                nc.tensor.matmul(ps_a[nb][:], wa[:], rhs,
                                                 start=(kt == 0),
                                                 stop=(kt == HT - 1))
                                nc.tensor.matmul(ps_b[nb][:], wb[:], rhs,
                                                 start=(kt == 0),
                                                 stop=(kt == HT - 1))
                        for nb in range(NB):
                            sa = w7.tile([P, 512], f32, tag="sa")
                            nc.scalar.activation(sa[:], ps_a[nb][:], AF.Silu)
                            ht = w7.tile([P, 512], f32r, tag="ht")
                            nc.vector.tensor_mul(ht[:], sa[:], ps_b[nb][:])
                            nc.sync.dma_start(
                                h_dram[P * t:P * (t + 1),
                                       t0 + 512 * nb:t0 + 512 * (nb + 1)],
                                ht[:])

                # MLP2 + residual eviction
                with ExitStack() as s8:
                    hp = s8.enter_context(tc.tile_pool(name="hpool", bufs=1))
                    h_t = hp.tile([P, FT, HW_], f32r, tag="h_t")
                    nc.sync.dma_start(
                        h_t[:],
                        h_dram.rearrange("(ft p) tt -> p ft tt",
                                         p=P)[:, :, t0:t0 + HW_])
                    w8 = s8.enter_context(tc.tile_pool(name="p8w", bufs=4))
                    wst8 = s8.enter_context(tc.tile_pool(name="w2st", bufs=6))
                    ps8 = s8.enter_context(
                        tc.tile_pool(name="p8ps", bufs=4, space="PSUM"))
                    for m in range(HT):
                        for nb in range(NB):
                            pp = ps8.tile([P, 512], f32, tag="pp8")
                            for kt in range(FT):
                                w2t = wst8.tile([P, P], f32r, tag="w2t")
                                nc.sync.dma_start(
                                    w2t[:], w2T[P * kt:P * (kt + 1),
                                                P * m:P * (m + 1)])
                                nc.tensor.matmul(
                                    pp[:], w2t[:],
                                    h_t[:, kt, 512 * nb:512 * (nb + 1)],
                                    start=(kt == 0), stop=(kt == FT - 1))
                            hmb = w8.tile([P, 512], f32, tag="hmb8")
                            nc.sync.dma_start(
                                hmb[:],
                                hm_dram[P * m:P * (m + 1),
                                        t0 + 512 * nb:t0 + 512 * (nb + 1)])
                            ev = w8.tile([P, 512], f32, tag="ev8")
                            nc.vector.scalar_tensor_tensor(
                                ev[:], hmb[:], 1.0 / N_CORES, pp[:],
                                OP.mult, OP.add)
                            nc.sync.dma_start(
                                outT[P * m:P * (m + 1),
                                     t0 + 512 * nb:t0 + 512 * (nb + 1)],
                                ev[:])


def _prepare_inputs(positions, hidden_states, ln1_w, wqkv, bqkv, wo, ln2_w,
                    w_h_to_4h, w_4h_to_h):
    """Shard + lay out inputs for the 8 cores. Returns list of in_maps."""
    positions = np.asarray(positions).astype(np.int64)
    hidden = np.asarray(hidden_states, dtype=np.float32)
    ln1_w = np.asarray(ln1_w, dtype=np.float32)
    ln2_w = np.asarray(ln2_w, dtype=np.float32)
    wqkv = np.asarray(wqkv, dtype=np.float32)
    bqkv = np.asarray(bqkv, dtype=np.float32)
    wo = np.asarray(wo, dtype=np.float32)
    w1 = np.asarray(w_h_to_4h, dtype=np.float32)
    w2 = np.asarray(w_4h_to_h, dtype=np.float32)

    hidT = np.ascontiguousarray(hidden.reshape(T, H).T)    # [H, T]

    # rope tables [64, T]
    pos = positions.reshape(T).astype(np.float64)
    inv_freq = 1.0 / (ROPE_BASE ** (np.arange(64, dtype=np.float64) / 64.0))
    ang = inv_freq[:, None] * pos[None, :]
    cosT = np.concatenate([np.cos(ang), np.cos(ang)], axis=0).astype(np.float32)
    sinT = np.concatenate([np.sin(ang), np.sin(ang)], axis=0).astype(np.float32)

    # shifted causal masks for the 4 diagonal sub-blocks [P, 4*CHUNK]
    tk = np.arange(P)[:, None]
    tq = np.arange(CHUNK)[None, :]
    maskT = np.concatenate(
        [(tk + P * o <= tq).astype(np.float32) for o in range(4)], axis=1)

    scale = 1.0 / math.sqrt(D)
    in_maps = []
    for c in range(N_CORES):
        g = c // 4
        q_rows = slice(512 * c, 512 * (c + 1))
        k_rows = slice(NH * D + g * D, NH * D + (g + 1) * D)
        v_rows = slice((NH + NKV) * D + g * D, (NH + NKV) * D + (g + 1) * D)
        wq_sh = np.concatenate([wqkv[q_rows] * scale, wqkv[k_rows],
                                wqkv[v_rows]], axis=0)      # [768, H]
        wq_sh = wq_sh * ln1_w[None, :]
        wqkvT_c = _round_tf32(np.ascontiguousarray(wq_sh.T))  # [H, 768]
        b_sh = np.concatenate([bqkv[q_rows] * scale, bqkv[k_rows],
                               bqkv[v_rows]])
        bqkvT_c = np.ascontiguousarray(b_sh.reshape(6, P).T)  # [P, 6]

        woT_c = _round_tf32(np.ascontiguousarray(wo[:, q_rows].T))  # [512, H]

        f_rows = slice(F_SH * c, F_SH * (c + 1))
        a_part = w1[f_rows] * ln2_w[None, :]                 # [1712, H]
        b_part = w1[FFN + F_SH * c:FFN + F_SH * (c + 1)] * ln2_w[None, :]
        pad = np.zeros((FP_SH - F_SH, H), np.float32)
        w1_sh = np.concatenate([a_part, pad, b_part, pad], axis=0)  # [3584, H]
        w1T_c = _round_tf32(np.ascontiguousarray(w1_sh.T))   # [H, 3584]

        w2_sh = w2[:, f_rows]                                # [H, 1712]
        w2T_c = np.zeros((FP_SH, H), np.float32)
        w2T_c[:F_SH] = w2_sh.T
        w2T_c = _round_tf32(w2T_c)                           # [1792, H]

        in_maps.append({
            "hidT": _round_tf32(hidT), "cosT": cosT, "sinT": sinT, "maskT": maskT,
            "wqkvT": wqkvT_c, "bqkvT": bqkvT_c, "woT": woT_c,
            "w1T": w1T_c, "w2T": w2T_c,
        })
    return in_maps


def kernel(**inputs):
    if "nc" not in _CACHE:
        _CACHE["nc"] = _build_program()
    nc = _CACHE["nc"]
    in_maps = _prepare_inputs(**inputs)
    res = bass_utils.run_bass_kernel_spmd(nc, in_maps,
                                          core_ids=list(range(N_CORES)))
    acc = res.results[0]["outT"].astype(np.float32).copy()
    for c in range(1, N_CORES):
        acc += res.results[c]["outT"]
    out = np.ascontiguousarray(acc.T).reshape(B, S, H)
    return out
